# revision 20
# baseline (speedup 1.0000x reference)
import sys, os
import numpy as np

for _p in ("/opt/trn_rl_repo", "/root/.axon_site/_ro/trn_rl_repo"):
    if os.path.isdir(_p) and _p not in sys.path:
        sys.path.insert(0, _p)

N_RADIAL = 5
N_BASIS = 7
R_MAX = 6.0
N_ATOMS = 10000
N_EDGES = 600000
N_SPECIES = 10

N_CORE = 8
WPC = 20          # windows per core
WA = 64           # atoms per window
APC = WPC * WA    # atoms per core (1280)
A_PAD = N_CORE * APC
T_TILES = WPC // 2  # atom tiles of 128 per core
CPW = 4           # windows per chunk
NCHUNK = WPC // CPW
NCOMP = 100       # 5 radial x 20 sym monomials
NOUT = 430        # device output cols per atom

BETTA = float((N_BASIS / R_MAX) ** 2)
SHIFTS = np.linspace(0.0, R_MAX, N_BASIS).astype(np.float32)

# symmetric monomial index maps (ascending order)
U2 = [(0, 0), (0, 1), (0, 2), (1, 1), (1, 2), (2, 2)]
U3 = [(0, 0, 0), (0, 0, 1), (0, 0, 2), (0, 1, 1), (0, 1, 2), (0, 2, 2),
      (1, 1, 1), (1, 1, 2), (1, 2, 2), (2, 2, 2)]
U2_IDX = {p: i for i, p in enumerate(U2)}
U3_IDX = {p: i for i, p in enumerate(U3)}
W2 = np.array([1, 2, 2, 1, 2, 1], dtype=np.float32)
W3 = np.array([1, 3, 3, 3, 6, 3, 1, 3, 3, 1], dtype=np.float32)
# full 3x3 -> unique map  (col j + 3*i)
M2MAP = [U2_IDX[tuple(sorted((i, j)))] for i in range(3) for j in range(3)]
# full 27 -> unique map (col k + 3*j + 9*i)
M3MAP = [U3_IDX[tuple(sorted((i, j, k)))]
         for i in range(3) for j in range(3) for k in range(3)]


def _tril_2d(n):
    return np.array([[i, j] for i in range(n) for j in range(i + 1)], dtype=np.int32)


def _tril_3d(n):
    return np.array([[i, j, k] for i in range(n) for j in range(i + 1) for k in range(j + 1)], dtype=np.int32)


def _affine_runs(srclist):
    """Greedy split of (dst contiguous, src arbitrary) into (dst0, src0, step, n) runs."""
    runs = []
    i = 0
    n = len(srclist)
    while i < n:
        j = i + 1
        if j < n and srclist[j] - srclist[i] >= 0:
            step = srclist[j] - srclist[i]
            while j + 1 < n and srclist[j + 1] - srclist[j] == step:
                j += 1
            runs.append((i, srclist[i], step, j - i + 1))
            i = j + 1
        else:
            runs.append((i, srclist[i], 1, 1))
            i += 1
    return runs


# ----------------------------------------------------------------------------
# host-side preprocessing: sort/shard/pad, build per-core device input arrays
# ----------------------------------------------------------------------------

def _prep(dr_vec, Z, neighbor_idxs, W):
    dr_vec = np.asarray(dr_vec, dtype=np.float32)
    Z = np.asarray(Z).astype(np.int64)
    idx_i = np.asarray(neighbor_idxs[0]).astype(np.int64)
    idx_j = np.asarray(neighbor_idxs[1]).astype(np.int64)
    W = np.asarray(W, dtype=np.float32)

    deg = np.bincount(idx_j, minlength=N_ATOMS)
    # snake-assign atoms (by degree desc) to N_CORE*WPC windows for balance
    NW = N_CORE * WPC
    order = np.argsort(-deg, kind="stable")
    slot_atoms = [[] for _ in range(NW)]
    pos = 0
    direction = 1
    w = 0
    for a in order:
        slot_atoms[w].append(a)
        if direction == 1:
            if w == NW - 1:
                direction = -1
            else:
                w += 1
        else:
            if w == 0:
                direction = 1
            else:
                w -= 1
    # atom -> (window, local slot)
    slot_of_atom = np.full(N_ATOMS, -1, dtype=np.int64)
    atoms_of_slot = np.full((NW, WA), -1, dtype=np.int64)
    for wi in range(NW):
        al = slot_atoms[wi]
        assert len(al) <= WA, f"window {wi} has {len(al)} atoms > {WA}"
        for k, a in enumerate(al):
            slot_of_atom[a] = wi * WA + k
            atoms_of_slot[wi, k] = a

    win_edges = np.bincount(slot_of_atom[idx_j] // WA, minlength=NW)
    BPW = int(np.ceil(win_edges.max() / 128.0))
    NCOLS = WPC * BPW
    FW = CPW * BPW
    EPC = NCOLS * 128  # padded edges per core

    # per-edge derived data
    gslot = slot_of_atom[idx_j]
    core_of_edge = gslot // APC
    win_local = (gslot % APC) // WA
    a_loc = gslot % WA
    pid = Z[idx_i] * N_SPECIES + Z[idx_j]

    Wpair = W.reshape(N_SPECIES * N_SPECIES, N_RADIAL * N_BASIS).astype(np.float16)

    # flattened edge buffers per core, window-major with padding
    x_all = np.zeros((N_CORE, NCOLS, 128), dtype=np.float32)
    y_all = np.zeros((N_CORE, NCOLS, 128), dtype=np.float32)
    z_all = np.zeros((N_CORE, NCOLS, 128), dtype=np.float32)
    x_all[:] = 100.0  # dummy edges: dr=100 -> cutoff 0
    al_all = np.zeros((N_CORE, NCOLS, 128), dtype=np.float16)
    cf_all = np.zeros((N_CORE, NCOLS, 128, 35), dtype=np.float16)

    eorder = np.argsort(gslot, kind="stable")
    eg = eorder  # edges sorted by global slot => grouped by (core, window)
    gs_sorted = gslot[eg]
    # boundaries per window
    win_of_sorted = gs_sorted // WA
    starts = np.searchsorted(win_of_sorted, np.arange(NW))
    ends = np.searchsorted(win_of_sorted, np.arange(NW), side="right")
    for wi in range(NW):
        c = wi // WPC
        wl = wi % WPC
        es = eg[starts[wi]:ends[wi]]
        cnt = len(es)
        col0 = wl * BPW
        # slot positions: fill columns [col0, col0+BPW), partition-major
        lin = np.arange(cnt)
        cols = col0 + lin // 128
        parts = lin % 128
        x_all[c, cols, parts] = dr_vec[es, 0]
        y_all[c, cols, parts] = dr_vec[es, 1]
        z_all[c, cols, parts] = dr_vec[es, 2]
        al_all[c, cols, parts] = a_loc[es].astype(np.float16)
        cf_all[c, cols, parts, :] = Wpair[pid[es]]

    # device layouts: [NCHUNK, 128, planes*FW]
    def to_chunks(arr_cols_part, nplanes):
        # arr: [NCORE, NCOLS, 128] or [NCORE, NCOLS, 128, P]
        if nplanes == 1:
            a = arr_cols_part.reshape(N_CORE, NCHUNK, FW, 128)
            a = np.transpose(a, (0, 1, 3, 2))  # [C, ch, 128, FW]
            return np.ascontiguousarray(a)
        else:
            a = arr_cols_part.reshape(N_CORE, NCHUNK, FW, 128, nplanes)
            a = np.transpose(a, (0, 1, 3, 4, 2))  # [C, ch, 128, P, FW]
            return np.ascontiguousarray(a.reshape(N_CORE, NCHUNK, 128, nplanes * FW))

    xs = to_chunks(x_all, 1)
    ys = to_chunks(y_all, 1)
    zs = to_chunks(z_all, 1)
    als = np.ascontiguousarray(np.repeat(to_chunks(al_all, 1), 2, axis=3))
    cfs = to_chunks(cf_all, 35)

    xyz = np.concatenate([xs[..., None, :].reshape(N_CORE, NCHUNK, 128, 1, FW),
                          ys.reshape(N_CORE, NCHUNK, 128, 1, FW),
                          zs.reshape(N_CORE, NCHUNK, 128, 1, FW)], axis=3)
    xyz = np.ascontiguousarray(xyz.reshape(N_CORE, NCHUNK, 128, 3 * FW))

    iota64 = np.broadcast_to(np.arange(WA, dtype=np.float16), (128, WA)).copy()
    shifts_t = np.broadcast_to(SHIFTS, (128, N_BASIS)).copy()
    wconst = np.broadcast_to(np.concatenate([W2, W3]), (128, 16)).copy().astype(np.float16)

    meta = dict(BPW=BPW, FW=FW, NCOLS=NCOLS,
                atoms_of_slot=atoms_of_slot)
    ins = dict(xyz=xyz, coef=cfs, al=als, iota=iota64, shifts=shifts_t, wconst=wconst)
    return ins, meta


# ----------------------------------------------------------------------------
# bass program
# ----------------------------------------------------------------------------

def _build_program(BPW):
    import concourse.bass as bass
    import concourse.mybir as mybir
    from concourse import tile, bacc

    FW = CPW * BPW
    f32 = mybir.dt.float32
    f16 = mybir.dt.float16

    nc = bacc.Bacc("TRN2", target_bir_lowering=False, debug=False,
                   num_devices=N_CORE)
    # extra const AP for Sin bias
    _pi2 = float(np.pi / 2)
    _ct = nc.alloc_sbuf_tensor("const-pi2", [128, 1], f32)
    nc.gpsimd.memset(_ct.ap(), _pi2)
    nc.const_aps.aps[(f32, _pi2)] = _ct.ap()
    nc.all_engine_barrier()
    xyz_d = nc.dram_tensor("xyz", [NCHUNK, 128, 3 * FW], f32, kind="ExternalInput")
    coef_d = nc.dram_tensor("coef", [NCHUNK, 128, 35 * FW], f16, kind="ExternalInput")
    al_d = nc.dram_tensor("al", [NCHUNK, 128, 2 * FW], f16, kind="ExternalInput")
    iota_d = nc.dram_tensor("iota", [128, WA], f16, kind="ExternalInput")
    shifts_d = nc.dram_tensor("shifts", [128, N_BASIS], f32, kind="ExternalInput")
    wconst_d = nc.dram_tensor("wconst", [128, 16], f16, kind="ExternalInput")
    out_d = nc.dram_tensor("out", [128, T_TILES * NOUT], f32, kind="ExternalOutput")

    with tile.TileContext(nc) as tc:
        with (
            tc.tile_pool(name="const", bufs=1) as cpool,
            tc.tile_pool(name="inp", bufs=2) as ipool,
            tc.tile_pool(name="work", bufs=2) as wpool,
            tc.tile_pool(name="medge", bufs=2) as mpool,
            tc.tile_pool(name="acc", bufs=1) as apool,
            tc.tile_pool(name="psum", bufs=2, space="PSUM") as pspool,
            tc.tile_pool(name="phb", bufs=1) as bpool,
            tc.tile_pool(name="phbs", bufs=4) as spool,
        ):
            six_sb = cpool.tile([128, 1], f32)
            nc.vector.memset(six_sb[:], R_MAX)
            iota_sb = cpool.tile([128, WA], f16)
            nc.sync.dma_start(iota_sb[:], iota_d[:, :])
            shifts_sb = cpool.tile([128, N_BASIS], f32)
            nc.sync.dma_start(shifts_sb[:], shifts_d[:, :])
            wc_sb = cpool.tile([128, 16], f16)
            nc.sync.dma_start(wc_sb[:], wconst_d[:, :])

            M_sb = apool.tile([128, T_TILES * NCOMP], f16)

            for ch in range(NCHUNK):
                xyz_sb = ipool.tile([128, 3 * FW], f32, tag="xyz")
                nc.sync.dma_start(xyz_sb[:], xyz_d[ch, :, :])
                coef_sb = ipool.tile([128, 35 * FW], f16, tag="coef")
                nc.sync.dma_start(coef_sb[:], coef_d[ch, :, :])
                al_sb = ipool.tile([128, 2 * FW], f16, tag="al")
                nc.sync.dma_start(al_sb[:], al_d[ch, :, :])

                x = xyz_sb[:, 0:FW]
                y = xyz_sb[:, FW:2 * FW]
                z = xyz_sb[:, 2 * FW:3 * FW]

                t1 = wpool.tile([128, FW], f32, tag="t1")
                t2 = wpool.tile([128, FW], f32, tag="t2")
                nc.vector.tensor_tensor(t1[:], x, x, op=mybir.AluOpType.mult)
                nc.vector.tensor_tensor(t2[:], y, y, op=mybir.AluOpType.mult)
                nc.vector.tensor_tensor(t1[:], t1[:], t2[:], op=mybir.AluOpType.add)
                nc.vector.tensor_tensor(t2[:], z, z, op=mybir.AluOpType.mult)
                nc.vector.tensor_tensor(t1[:], t1[:], t2[:], op=mybir.AluOpType.add)
                dr = wpool.tile([128, FW], f32, tag="dr")
                nc.scalar.sqrt(dr[:], t1[:])
                # inv = 1/(dr+1e-5)
                nc.scalar.activation(t2[:], dr[:],
                                     mybir.ActivationFunctionType.Copy,
                                     bias=1e-5)
                inv = wpool.tile([128, FW], f32, tag="inv")
                nc.vector.reciprocal(inv[:], t2[:])
                # cutoff = 0.5*sin(pi/6*min(dr,6) + pi/2) + 0.5   (fp16)
                nc.vector.tensor_tensor(
                    t2[:], dr[:],
                    six_sb[:].broadcast_to((128, FW)),
                    op=mybir.AluOpType.min)
                sinv = wpool.tile([128, FW], f16, tag="sinv")
                nc.scalar.activation(sinv[:], t2[:], mybir.ActivationFunctionType.Sin,
                                     bias=_pi2, scale=float(np.pi / R_MAX))
                cut = wpool.tile([128, FW], f16, tag="cut")
                nc.scalar.activation(cut[:], sinv[:],
                                     mybir.ActivationFunctionType.Copy,
                                     bias=0.5, scale=0.5)

                # mono: [128, 20*FW] planes: 1, dnx, dny, dnz, u2(6), u3(10)
                mono = wpool.tile([128, 20 * FW], f16, tag="mono")
                nc.vector.memset(mono[:, 0:FW], 1.0)
                dnx = mono[:, FW:2 * FW]
                dny = mono[:, 2 * FW:3 * FW]
                dnz = mono[:, 3 * FW:4 * FW]
                nc.vector.tensor_tensor(dnx, x, inv[:], op=mybir.AluOpType.mult)
                nc.vector.tensor_tensor(dny, y, inv[:], op=mybir.AluOpType.mult)
                nc.vector.tensor_tensor(dnz, z, inv[:], op=mybir.AluOpType.mult)
                dn3 = mono[:, FW:4 * FW].rearrange("p (c f) -> p c f", c=3)
                # u2 planes 4..9 : xx,xy,xz | yy,yz | zz
                u2v = mono[:, 4 * FW:10 * FW].rearrange("p (c f) -> p c f", c=6)
                nc.gpsimd.tensor_tensor(
                    u2v[:, 0:3, :],
                    dnx.unsqueeze(1).broadcast_to((128, 3, FW)),
                    dn3, op=mybir.AluOpType.mult)
                nc.gpsimd.tensor_tensor(
                    u2v[:, 3:5, :],
                    dny.unsqueeze(1).broadcast_to((128, 2, FW)),
                    dn3[:, 1:3, :], op=mybir.AluOpType.mult)
                nc.gpsimd.tensor_tensor(
                    u2v[:, 5:6, :],
                    dnz.unsqueeze(1).broadcast_to((128, 1, FW)),
                    dn3[:, 2:3, :], op=mybir.AluOpType.mult)
                # u3 planes 10..19
                u3v = mono[:, 10 * FW:20 * FW].rearrange("p (c f) -> p c f", c=10)
                nc.gpsimd.tensor_tensor(
                    u3v[:, 0:3, :],
                    u2v[:, 0:1, :].broadcast_to((128, 3, FW)),
                    dn3, op=mybir.AluOpType.mult)
                nc.gpsimd.tensor_tensor(
                    u3v[:, 3:5, :],
                    u2v[:, 1:2, :].broadcast_to((128, 2, FW)),
                    dn3[:, 1:3, :], op=mybir.AluOpType.mult)
                nc.gpsimd.tensor_tensor(
                    u3v[:, 5:6, :],
                    u2v[:, 2:3, :].broadcast_to((128, 1, FW)),
                    dn3[:, 2:3, :], op=mybir.AluOpType.mult)
                nc.gpsimd.tensor_tensor(
                    u3v[:, 6:8, :],
                    u2v[:, 3:4, :].broadcast_to((128, 2, FW)),
                    dn3[:, 1:3, :], op=mybir.AluOpType.mult)
                nc.gpsimd.tensor_tensor(
                    u3v[:, 8:9, :],
                    u2v[:, 4:5, :].broadcast_to((128, 1, FW)),
                    dn3[:, 2:3, :], op=mybir.AluOpType.mult)
                nc.gpsimd.tensor_tensor(
                    u3v[:, 9:10, :],
                    u2v[:, 5:6, :].broadcast_to((128, 1, FW)),
                    dn3[:, 2:3, :], op=mybir.AluOpType.mult)

                # basis: g = exp(-betta*(dr-s)^2)  [128, 7*FW] fp16
                diff = wpool.tile([128, N_BASIS * FW], f16, tag="diff")
                dv = diff[:].rearrange("p (b f) -> p b f", b=N_BASIS)
                nc.vector.tensor_tensor(
                    dv,
                    dr[:].unsqueeze(1).broadcast_to((128, N_BASIS, FW)),
                    shifts_sb[:].unsqueeze(2).broadcast_to((128, N_BASIS, FW)),
                    op=mybir.AluOpType.subtract)
                g = wpool.tile([128, N_BASIS * FW], f16, tag="g")
                gv = g[:].rearrange("p (b f) -> p b f", b=N_BASIS)
                nc.vector.tensor_tensor(gv, dv, dv, op=mybir.AluOpType.mult)
                nc.scalar.activation(g[:], g[:], mybir.ActivationFunctionType.Exp,
                                     scale=-BETTA)

                # rad[r] = cutoff * sum_b coef[r,b]*g[b]    [128, 5*FW] fp16
                coefv = coef_sb[:].rearrange("p (r b f) -> p r b f", r=N_RADIAL, b=N_BASIS)
                radA = wpool.tile([128, N_RADIAL * FW], f16, tag="radA")
                radB = wpool.tile([128, N_RADIAL * FW], f16, tag="radB")
                rAv = radA[:].rearrange("p (r f) -> p r f", r=N_RADIAL)
                rBv = radB[:].rearrange("p (r f) -> p r f", r=N_RADIAL)
                for b in range(N_BASIS):
                    gb = gv[:, b, :].unsqueeze(1).broadcast_to((128, N_RADIAL, FW))
                    tgt = rAv if b == 0 else rBv
                    nc.vector.tensor_tensor(tgt, coefv[:, :, b, :], gb,
                                            op=mybir.AluOpType.mult)
                    if b > 0:
                        nc.vector.tensor_tensor(rAv, rAv, rBv, op=mybir.AluOpType.add)
                radf = wpool.tile([128, N_RADIAL * FW], f16, tag="radf")
                rfv = radf[:].rearrange("p (r f) -> p r f", r=N_RADIAL)
                nc.vector.tensor_tensor(
                    rfv, rAv,
                    cut[:].unsqueeze(1).broadcast_to((128, N_RADIAL, FW)),
                    op=mybir.AluOpType.mult)

                # medge [128, 100*FW] = rad (r) x mono (t)
                medge = mpool.tile([128, NCOMP * FW], f16, tag="medge")
                mev = medge[:].rearrange("p (r t f) -> p r t f", r=N_RADIAL, t=20)
                nc.vector.tensor_tensor(
                    mev,
                    rfv.unsqueeze(2).broadcast_to((128, N_RADIAL, 20, FW)),
                    mono[:].rearrange("p (t f) -> p t f", t=20)
                        .unsqueeze(1).broadcast_to((128, N_RADIAL, 20, FW)),
                    op=mybir.AluOpType.mult)

                # onehot [128, FW*WA]
                oh = mpool.tile([128, FW * WA], f16, tag="oh")
                ohv = oh[:].rearrange("p (f c) -> p f c", c=WA)
                oh4 = oh[:].rearrange("p (f c q) -> p f c q", c=WA // 2, q=2)
                nc.vector.tensor_tensor(
                    oh4,
                    iota_sb[:].rearrange("p (c q) -> p c q", q=2)
                        .unsqueeze(1).broadcast_to((128, FW, WA // 2, 2)),
                    al_sb[:].rearrange("p (f q) -> p f q", q=2)
                        .unsqueeze(2).broadcast_to((128, FW, WA // 2, 2)),
                    op=mybir.AluOpType.is_equal)

                # segment-sum matmuls: 4 windows in this chunk
                mev2 = medge[:].rearrange("p (c f) -> p c f", c=NCOMP)
                for w4 in range(CPW):
                    w = ch * CPW + w4
                    pair = w // 2
                    hi = w % 2
                    if hi == 0:
                        ps = pspool.tile([128, NCOMP], f32, tag="ps")
                    for b in range(BPW):
                        col = w4 * BPW + b
                        nc.tensor.matmul(
                            ps[hi * WA:(hi + 1) * WA, :],
                            ohv[:, col, :],
                            mev2[:, :, col],
                            start=(b == 0), stop=(b == BPW - 1),
                            tile_position=(0, hi * WA),
                        )
                    if hi == 1:
                        nc.scalar.copy(
                            M_sb[:, pair * NCOMP:(pair + 1) * NCOMP], ps[:, :])

            # ----------------- phase B: per-atom contractions -----------------
            _lp = nc.allow_low_precision("f16 phase-B scratch; final out is f32")
            _lp.__enter__()
            # NOTE: TensorTensor ISA allows at most 3 free dims per operand and
            # broadcast dims never merge -> split every 4-dim product in python.
            T = T_TILES
            Mv = M_sb[:].rearrange("p (T r q) -> p T r q", T=T, r=N_RADIAL)
            m1 = Mv[:, :, :, 1:4]
            s2 = Mv[:, :, :, 4:10]
            s3 = Mv[:, :, :, 10:20]

            out_sb = bpool.tile([128, T * NOUT], f32)
            outv = out_sb[:].rearrange("p (T c) -> p T c", T=T)

            mul = mybir.AluOpType.mult
            addop = mybir.AluOpType.add
            X = mybir.AxisListType.X

            # m0
            nc.vector.tensor_copy(outv[:, :, 0:5], Mv[:, :, :, 0])

            # weighted s2w, s3w
            s2w = bpool.tile([128, T * 5 * 6], f16)
            s2wv = s2w[:].rearrange("p (T r u) -> p T r u", T=T, r=5)
            nc.vector.tensor_tensor(
                s2wv, s2,
                wc_sb[:, 0:6].unsqueeze(1).unsqueeze(1).broadcast_to((128, T, 5, 6)),
                op=mul)
            s3w = bpool.tile([128, T * 5 * 10], f16)
            s3wv = s3w[:].rearrange("p (T r u) -> p T r u", T=T, r=5)
            nc.vector.tensor_tensor(
                s3wv, s3,
                wc_sb[:, 6:16].unsqueeze(1).unsqueeze(1).broadcast_to((128, T, 5, 10)),
                op=mul)

            # c1/c2/c3 grams (split per s)
            for (lhs, rhs_, width, off) in ((m1, m1, 3, 5), (s2wv, s2, 6, 30), (s3wv, s3, 10, 55)):
                for s in range(5):
                    pb = spool.tile([128, T * 5 * 10], f16, tag="pb")
                    pv = pb[:, 0:T * 5 * width].rearrange(
                        "p (T r u) -> p T r u", T=T, r=5)
                    nc.vector.tensor_tensor(
                        pv, lhs,
                        rhs_[:, :, s, :].unsqueeze(2).broadcast_to((128, T, 5, width)),
                        op=mul)
                    nc.vector.tensor_reduce(
                        outv[:, :, off + s:off + 25:5], pv, axis=X, op=addop)

            # m2full [128, T*5*9]
            m2f = bpool.tile([128, T * 5 * 9], f16)
            m2fv = m2f[:].rearrange("p (T r c) -> p T r c", T=T, r=5)
            for (d0, s0, st, n) in _affine_runs(M2MAP):
                nc.vector.tensor_copy(m2fv[:, :, :, d0:d0 + n],
                                      s2[:, :, :, s0:s0 + st * (n - 1) + 1:st] if st != 0 else
                                      s2[:, :, :, s0:s0 + 1].broadcast_to((128, T, 5, n)))
            # m3full [128, T*5*27]
            m3f = bpool.tile([128, T * 5 * 27], f16)
            m3fv = m3f[:].rearrange("p (T r c) -> p T r c", T=T, r=5)
            for (d0, s0, st, n) in _affine_runs(M3MAP):
                nc.vector.tensor_copy(m3fv[:, :, :, d0:d0 + n],
                                      s3[:, :, :, s0:s0 + st * (n - 1) + 1:st] if st != 0 else
                                      s3[:, :, :, s0:s0 + 1].broadcast_to((128, T, 5, n)))

            # c5 (cols 80:155): c5[r,s,t] = sum_ij m1[r,i] m1[s,j] m2f[t,ij]
            for r in range(5):
                for s in range(r + 1):
                    pair = r * (r + 1) // 2 + s
                    p1b = spool.tile([128, T * 9], f16, tag="p1b")
                    qb = spool.tile([128, T * 5 * 9], f16, tag="qb")
                    p1v = p1b[:].rearrange("p (T u) -> p T u", T=T)
                    p1v9 = p1b[:].rearrange("p (T i j) -> p T i j", T=T, i=3)
                    nc.vector.tensor_tensor(
                        p1v9,
                        m1[:, :, r, :].unsqueeze(3).broadcast_to((128, T, 3, 3)),
                        m1[:, :, s, :].unsqueeze(2).broadcast_to((128, T, 3, 3)),
                        op=mul)
                    qv = qb[:].rearrange("p (T t u) -> p T t u", T=T, t=5)
                    nc.vector.tensor_tensor(
                        qv,
                        p1v.unsqueeze(2).broadcast_to((128, T, 5, 9)),
                        m2fv, op=mul)
                    nc.vector.tensor_reduce(
                        outv[:, :, 80 + pair * 5:80 + pair * 5 + 5],
                        qv, axis=X, op=addop)

            # c4 (cols 155:230): A_rs(j,k) = sum_i m2f[r](i,j) m2f[s](i,k)
            for r in range(5):
                m2fr = m2fv[:, :, r, :].rearrange("p T (i j) -> p T i j", i=3)
                for s in range(r + 1):
                    pair = r * (r + 1) // 2 + s
                    ab = spool.tile([128, T * 9], f16, tag="ab")
                    abv = ab[:].rearrange("p (T j k) -> p T j k", T=T, j=3)
                    qb = spool.tile([128, T * 5 * 9], f16, tag="qb")
                    m2fs = m2fv[:, :, s, :].rearrange("p T (i k) -> p T i k", i=3)
                    for j in range(3):
                        tb = spool.tile([128, T * 27], f16, tag="tb")
                        tv = tb[:, 0:T * 3 * 3].rearrange(
                            "p (T k i) -> p T k i", T=T, k=3)
                        nc.gpsimd.tensor_tensor(
                            tv,
                            m2fr[:, :, :, j].unsqueeze(2).broadcast_to((128, T, 3, 3)),
                            m2fs.rearrange("p T i k -> p T k i"),
                            op=mul)
                        nc.vector.tensor_reduce(
                            abv[:, :, j, :], tv, axis=X, op=addop)
                    qv = qb[:].rearrange("p (T t u) -> p T t u", T=T, t=5)
                    nc.vector.tensor_tensor(
                        qv,
                        ab[:].rearrange("p (T u) -> p T u", T=T)
                            .unsqueeze(2).broadcast_to((128, T, 5, 9)),
                        m2fv, op=mul)
                    nc.vector.tensor_reduce(
                        outv[:, :, 155 + pair * 5:155 + pair * 5 + 5],
                        qv, axis=X, op=addop)

            # c6 (cols 230:305): B_rs(k,l) = sum_ij m3f[r](ij,k) m3f[s](ij,l)
            for r in range(5):
                m3fr = m3fv[:, :, r, :].rearrange("p T (u k) -> p T u k", u=9)
                for s in range(r + 1):
                    pair = r * (r + 1) // 2 + s
                    ab = spool.tile([128, T * 9], f16, tag="ab")
                    abv = ab[:].rearrange("p (T j k) -> p T j k", T=T, j=3)
                    qb = spool.tile([128, T * 5 * 9], f16, tag="qb")
                    m3fs = m3fv[:, :, s, :].rearrange("p T (u l) -> p T u l", u=9)
                    for k in range(3):
                        tb = spool.tile([128, T * 27], f16, tag="tb")
                        tv = tb[:, 0:T * 3 * 9].rearrange(
                            "p (T l u) -> p T l u", T=T, l=3)
                        nc.gpsimd.tensor_tensor(
                            tv,
                            m3fr[:, :, :, k].unsqueeze(2).broadcast_to((128, T, 3, 9)),
                            m3fs.rearrange("p T u l -> p T l u"),
                            op=mul)
                        nc.vector.tensor_reduce(
                            abv[:, :, k, :], tv, axis=X, op=addop)
                    qv = qb[:].rearrange("p (T t u) -> p T t u", T=T, t=5)
                    nc.vector.tensor_tensor(
                        qv,
                        ab[:].rearrange("p (T u) -> p T u", T=T)
                            .unsqueeze(2).broadcast_to((128, T, 5, 9)),
                        m2fv, op=mul)
                    nc.vector.tensor_reduce(
                        outv[:, :, 230 + pair * 5:230 + pair * 5 + 5],
                        qv, axis=X, op=addop)

            # c7 (cols 305:430): C_rs(k) = sum_ij m3f[r](ij,k) m2f[s](ij)
            for r in range(5):
                m3fr = m3fv[:, :, r, :].rearrange("p T (u k) -> p T k u", u=9)
                for s in range(5):
                    tb = spool.tile([128, T * 27], f16, tag="tb")
                    cb = spool.tile([128, T * 5 * 3], f16, tag="cb")
                    qb = spool.tile([128, T * 5 * 9], f16, tag="qb")
                    tv = tb[:, 0:T * 3 * 9].rearrange(
                        "p (T k u) -> p T k u", T=T, k=3)
                    nc.gpsimd.tensor_tensor(
                        tv, m3fr,
                        m2fv[:, :, s, :].unsqueeze(2).broadcast_to((128, T, 3, 9)),
                        op=mul)
                    nc.vector.tensor_reduce(
                        cb[:, 0:T * 3].rearrange("p (T k) -> p T k", T=T),
                        tv, axis=X, op=addop)
                    qv = qb[:, 0:T * 5 * 3].rearrange(
                        "p (T t k) -> p T t k", T=T, t=5)
                    nc.vector.tensor_tensor(
                        qv,
                        cb[:, 0:T * 3].rearrange("p (T k) -> p T k", T=T)
                            .unsqueeze(2).broadcast_to((128, T, 5, 3)),
                        m1, op=mul)
                    nc.vector.tensor_reduce(
                        outv[:, :, 305 + r * 25 + s * 5:305 + r * 25 + s * 5 + 5],
                        qv, axis=X, op=addop)

            nc.sync.dma_start(out_d[:, :], out_sb[:])
            _lp.__exit__(None, None, None)

    nc.compile()
    return nc


_CACHE = {}


def kernel(dr_vec, Z, neighbor_idxs, W):
    from concourse.bass_utils import run_bass_kernel_spmd

    ins, meta = _prep(dr_vec, Z, neighbor_idxs, W)
    BPW = meta["BPW"]
    if BPW not in _CACHE:
        _CACHE[BPW] = _build_program(BPW)
    nc = _CACHE[BPW]

    in_maps = []
    for c in range(N_CORE):
        in_maps.append({
            "xyz": ins["xyz"][c], "coef": ins["coef"][c], "al": ins["al"][c],
            "iota": ins["iota"], "shifts": ins["shifts"], "wconst": ins["wconst"],
        })
    global LAST_EXEC_NS, LAST_RUN_WALL
    _t0 = __import__("time").perf_counter()
    trace = bool(int(os.environ.get("BASS_KERNEL_TRACE", "0")))
    res = run_bass_kernel_spmd(nc, in_maps, core_ids=list(range(N_CORE)),
                               trace=trace)
    LAST_RUN_WALL = __import__("time").perf_counter() - _t0
    LAST_EXEC_NS = res.exec_time_ns

    # assemble
    atoms_of_slot = meta["atoms_of_slot"]  # [NW, WA]
    O = np.zeros((A_PAD, NOUT), dtype=np.float32)
    for c in range(N_CORE):
        o = res.results[c]["out"].reshape(128, T_TILES, NOUT)
        o = np.transpose(o, (1, 0, 2)).reshape(APC, NOUT)
        O[c * APC:(c + 1) * APC] = o

    full = np.zeros((N_ATOMS, NOUT), dtype=np.float32)
    slots = atoms_of_slot.reshape(-1)
    valid = slots >= 0
    full[slots[valid]] = O[valid]

    t2 = _tril_2d(N_RADIAL)
    t3 = _tril_3d(N_RADIAL)
    pair_idx = (t2[:, 0] * (t2[:, 0] + 1)) // 2 + t2[:, 1]
    out = np.empty((N_ATOMS, 360), dtype=np.float32)
    out[:, 0:5] = full[:, 0:5]
    c1 = full[:, 5:30].reshape(-1, 5, 5)
    c2 = full[:, 30:55].reshape(-1, 5, 5)
    c3 = full[:, 55:80].reshape(-1, 5, 5)
    out[:, 5:20] = c1[:, t2[:, 0], t2[:, 1]]
    out[:, 20:35] = c2[:, t2[:, 0], t2[:, 1]]
    out[:, 35:50] = c3[:, t2[:, 0], t2[:, 1]]
    c4 = full[:, 155:230].reshape(-1, 15, 5)
    p3 = (t3[:, 0] * (t3[:, 0] + 1)) // 2 + t3[:, 1]
    out[:, 50:85] = c4[:, p3, t3[:, 2]]
    out[:, 85:160] = full[:, 80:155]
    out[:, 160:235] = full[:, 230:305]
    out[:, 235:360] = full[:, 305:430]
    return out


# revision 21
# speedup vs baseline: 1.1187x; 1.1187x over previous
import sys, os
import numpy as np

for _p in ("/opt/trn_rl_repo", "/root/.axon_site/_ro/trn_rl_repo"):
    if os.path.isdir(_p) and _p not in sys.path:
        sys.path.insert(0, _p)

N_RADIAL = 5
N_BASIS = 7
R_MAX = 6.0
N_ATOMS = 10000
N_EDGES = 600000
N_SPECIES = 10

N_CORE = 8
WPC = 20          # windows per core
WA = 64           # atoms per window
APC = WPC * WA    # atoms per core (1280)
A_PAD = N_CORE * APC
T_TILES = WPC // 2  # atom tiles of 128 per core
CPW = 4           # windows per chunk
NCHUNK = WPC // CPW
NCOMP = 100       # 5 radial x 20 sym monomials
NOUT = 430        # device output cols per atom

BETTA = float((N_BASIS / R_MAX) ** 2)
SHIFTS = np.linspace(0.0, R_MAX, N_BASIS).astype(np.float32)

# symmetric monomial index maps (ascending order)
U2 = [(0, 0), (0, 1), (0, 2), (1, 1), (1, 2), (2, 2)]
U3 = [(0, 0, 0), (0, 0, 1), (0, 0, 2), (0, 1, 1), (0, 1, 2), (0, 2, 2),
      (1, 1, 1), (1, 1, 2), (1, 2, 2), (2, 2, 2)]
U2_IDX = {p: i for i, p in enumerate(U2)}
U3_IDX = {p: i for i, p in enumerate(U3)}
W2 = np.array([1, 2, 2, 1, 2, 1], dtype=np.float32)
W3 = np.array([1, 3, 3, 3, 6, 3, 1, 3, 3, 1], dtype=np.float32)
# full 3x3 -> unique map  (col j + 3*i)
M2MAP = [U2_IDX[tuple(sorted((i, j)))] for i in range(3) for j in range(3)]
# full 27 -> unique map (col k + 3*j + 9*i)
M3MAP = [U3_IDX[tuple(sorted((i, j, k)))]
         for i in range(3) for j in range(3) for k in range(3)]


def _tril_2d(n):
    return np.array([[i, j] for i in range(n) for j in range(i + 1)], dtype=np.int32)


def _tril_3d(n):
    return np.array([[i, j, k] for i in range(n) for j in range(i + 1) for k in range(j + 1)], dtype=np.int32)


def _affine_runs(srclist):
    """Greedy split of (dst contiguous, src arbitrary) into (dst0, src0, step, n) runs."""
    runs = []
    i = 0
    n = len(srclist)
    while i < n:
        j = i + 1
        if j < n and srclist[j] - srclist[i] >= 0:
            step = srclist[j] - srclist[i]
            while j + 1 < n and srclist[j + 1] - srclist[j] == step:
                j += 1
            runs.append((i, srclist[i], step, j - i + 1))
            i = j + 1
        else:
            runs.append((i, srclist[i], 1, 1))
            i += 1
    return runs


# ----------------------------------------------------------------------------
# host-side preprocessing: sort/shard/pad, build per-core device input arrays
# ----------------------------------------------------------------------------

def _prep(dr_vec, Z, neighbor_idxs, W):
    dr_vec = np.asarray(dr_vec, dtype=np.float32)
    Z = np.asarray(Z).astype(np.int64)
    idx_i = np.asarray(neighbor_idxs[0]).astype(np.int64)
    idx_j = np.asarray(neighbor_idxs[1]).astype(np.int64)
    W = np.asarray(W, dtype=np.float32)

    deg = np.bincount(idx_j, minlength=N_ATOMS)
    # snake-assign atoms (by degree desc) to N_CORE*WPC windows for balance
    NW = N_CORE * WPC
    order = np.argsort(-deg, kind="stable")
    slot_atoms = [[] for _ in range(NW)]
    pos = 0
    direction = 1
    w = 0
    for a in order:
        slot_atoms[w].append(a)
        if direction == 1:
            if w == NW - 1:
                direction = -1
            else:
                w += 1
        else:
            if w == 0:
                direction = 1
            else:
                w -= 1
    # atom -> (window, local slot)
    slot_of_atom = np.full(N_ATOMS, -1, dtype=np.int64)
    atoms_of_slot = np.full((NW, WA), -1, dtype=np.int64)
    for wi in range(NW):
        al = slot_atoms[wi]
        assert len(al) <= WA, f"window {wi} has {len(al)} atoms > {WA}"
        for k, a in enumerate(al):
            slot_of_atom[a] = wi * WA + k
            atoms_of_slot[wi, k] = a

    win_edges = np.bincount(slot_of_atom[idx_j] // WA, minlength=NW)
    BPW = int(np.ceil(win_edges.max() / 128.0))
    NCOLS = WPC * BPW
    FW = CPW * BPW
    EPC = NCOLS * 128  # padded edges per core

    # per-edge derived data
    gslot = slot_of_atom[idx_j]
    core_of_edge = gslot // APC
    win_local = (gslot % APC) // WA
    a_loc = gslot % WA
    pid = Z[idx_i] * N_SPECIES + Z[idx_j]

    Wpair = W.reshape(N_SPECIES * N_SPECIES, N_RADIAL * N_BASIS).astype(np.float16)

    # flattened edge buffers per core, window-major with padding
    x_all = np.zeros((N_CORE, NCOLS, 128), dtype=np.float32)
    y_all = np.zeros((N_CORE, NCOLS, 128), dtype=np.float32)
    z_all = np.zeros((N_CORE, NCOLS, 128), dtype=np.float32)
    x_all[:] = 100.0  # dummy edges: dr=100 -> cutoff 0
    al_all = np.zeros((N_CORE, NCOLS, 128), dtype=np.float16)
    cf_all = np.zeros((N_CORE, NCOLS, 128, 35), dtype=np.float16)

    eorder = np.argsort(gslot, kind="stable")
    eg = eorder  # edges sorted by global slot => grouped by (core, window)
    gs_sorted = gslot[eg]
    # boundaries per window
    win_of_sorted = gs_sorted // WA
    starts = np.searchsorted(win_of_sorted, np.arange(NW))
    ends = np.searchsorted(win_of_sorted, np.arange(NW), side="right")
    for wi in range(NW):
        c = wi // WPC
        wl = wi % WPC
        es = eg[starts[wi]:ends[wi]]
        cnt = len(es)
        col0 = wl * BPW
        # slot positions: fill columns [col0, col0+BPW), partition-major
        lin = np.arange(cnt)
        cols = col0 + lin // 128
        parts = lin % 128
        x_all[c, cols, parts] = dr_vec[es, 0]
        y_all[c, cols, parts] = dr_vec[es, 1]
        z_all[c, cols, parts] = dr_vec[es, 2]
        al_all[c, cols, parts] = a_loc[es].astype(np.float16)
        cf_all[c, cols, parts, :] = Wpair[pid[es]]

    # device layouts: [NCHUNK, 128, planes*FW]
    def to_chunks(arr_cols_part, nplanes):
        # arr: [NCORE, NCOLS, 128] or [NCORE, NCOLS, 128, P]
        if nplanes == 1:
            a = arr_cols_part.reshape(N_CORE, NCHUNK, FW, 128)
            a = np.transpose(a, (0, 1, 3, 2))  # [C, ch, 128, FW]
            return np.ascontiguousarray(a)
        else:
            a = arr_cols_part.reshape(N_CORE, NCHUNK, FW, 128, nplanes)
            a = np.transpose(a, (0, 1, 3, 4, 2))  # [C, ch, 128, P, FW]
            return np.ascontiguousarray(a.reshape(N_CORE, NCHUNK, 128, nplanes * FW))

    xs = to_chunks(x_all, 1)
    ys = to_chunks(y_all, 1)
    zs = to_chunks(z_all, 1)
    als = np.ascontiguousarray(np.repeat(to_chunks(al_all, 1), 2, axis=3))
    cfs = to_chunks(cf_all, 35)

    xyz = np.concatenate([xs[..., None, :].reshape(N_CORE, NCHUNK, 128, 1, FW),
                          ys.reshape(N_CORE, NCHUNK, 128, 1, FW),
                          zs.reshape(N_CORE, NCHUNK, 128, 1, FW)], axis=3)
    xyz = np.ascontiguousarray(xyz.reshape(N_CORE, NCHUNK, 128, 3 * FW))

    iota64 = np.broadcast_to(np.arange(WA, dtype=np.float16), (128, WA)).copy()
    shifts_t = np.broadcast_to(SHIFTS, (128, N_BASIS)).copy()
    wconst = np.broadcast_to(np.concatenate([W2, W3]), (128, 16)).copy().astype(np.float16)

    meta = dict(BPW=BPW, FW=FW, NCOLS=NCOLS,
                atoms_of_slot=atoms_of_slot)
    ins = dict(xyz=xyz, coef=cfs, al=als, iota=iota64, shifts=shifts_t, wconst=wconst)
    return ins, meta


# ----------------------------------------------------------------------------
# bass program
# ----------------------------------------------------------------------------

def _build_program(BPW):
    import concourse.bass as bass
    import concourse.mybir as mybir
    from concourse import tile, bacc

    FW = CPW * BPW
    f32 = mybir.dt.float32
    f16 = mybir.dt.float16

    nc = bacc.Bacc("TRN2", target_bir_lowering=False, debug=False,
                   num_devices=N_CORE)
    # extra const AP for Sin bias
    _pi2 = float(np.pi / 2)
    _ct = nc.alloc_sbuf_tensor("const-pi2", [128, 1], f32)
    nc.gpsimd.memset(_ct.ap(), _pi2)
    nc.const_aps.aps[(f32, _pi2)] = _ct.ap()
    nc.all_engine_barrier()
    xyz_d = nc.dram_tensor("xyz", [NCHUNK, 128, 3 * FW], f32, kind="ExternalInput")
    coef_d = nc.dram_tensor("coef", [NCHUNK, 128, 35 * FW], f16, kind="ExternalInput")
    al_d = nc.dram_tensor("al", [NCHUNK, 128, 2 * FW], f16, kind="ExternalInput")
    iota_d = nc.dram_tensor("iota", [128, WA], f16, kind="ExternalInput")
    shifts_d = nc.dram_tensor("shifts", [128, N_BASIS], f32, kind="ExternalInput")
    wconst_d = nc.dram_tensor("wconst", [128, 16], f16, kind="ExternalInput")
    out_d = nc.dram_tensor("out", [128, T_TILES * NOUT], f32, kind="ExternalOutput")

    with tile.TileContext(nc) as tc:
        with (
            tc.tile_pool(name="const", bufs=1) as cpool,
            tc.tile_pool(name="inp", bufs=3) as ipool,
            tc.tile_pool(name="work", bufs=2) as wpool,
            tc.tile_pool(name="medge", bufs=2) as mpool,
            tc.tile_pool(name="acc", bufs=1) as apool,
            tc.tile_pool(name="psum", bufs=2, space="PSUM") as pspool,
            tc.tile_pool(name="phb", bufs=1) as bpool,
            tc.tile_pool(name="phbs", bufs=8) as spool,
        ):
            six_sb = cpool.tile([128, 1], f32)
            nc.vector.memset(six_sb[:], R_MAX)
            iota_sb = cpool.tile([128, WA], f16)
            nc.sync.dma_start(iota_sb[:], iota_d[:, :])
            shifts_sb = cpool.tile([128, N_BASIS], f32)
            nc.sync.dma_start(shifts_sb[:], shifts_d[:, :])
            wc_sb = cpool.tile([128, 16], f16)
            nc.sync.dma_start(wc_sb[:], wconst_d[:, :])

            M_sb = apool.tile([128, T_TILES * NCOMP], f16)

            for ch in range(NCHUNK):
                xyz_sb = ipool.tile([128, 3 * FW], f32, tag="xyz")
                nc.sync.dma_start(xyz_sb[:], xyz_d[ch, :, :])
                coef_sb = ipool.tile([128, 35 * FW], f16, tag="coef")
                nc.sync.dma_start(coef_sb[:], coef_d[ch, :, :])
                al_sb = ipool.tile([128, 2 * FW], f16, tag="al")
                nc.sync.dma_start(al_sb[:], al_d[ch, :, :])

                x = xyz_sb[:, 0:FW]
                y = xyz_sb[:, FW:2 * FW]
                z = xyz_sb[:, 2 * FW:3 * FW]

                t1 = wpool.tile([128, FW], f32, tag="t1")
                t2 = wpool.tile([128, FW], f32, tag="t2")
                nc.vector.tensor_tensor(t1[:], x, x, op=mybir.AluOpType.mult)
                nc.vector.tensor_tensor(t2[:], y, y, op=mybir.AluOpType.mult)
                nc.vector.tensor_tensor(t1[:], t1[:], t2[:], op=mybir.AluOpType.add)
                nc.vector.tensor_tensor(t2[:], z, z, op=mybir.AluOpType.mult)
                nc.vector.tensor_tensor(t1[:], t1[:], t2[:], op=mybir.AluOpType.add)
                dr = wpool.tile([128, FW], f32, tag="dr")
                nc.scalar.sqrt(dr[:], t1[:])
                # inv = 1/(dr+1e-5)
                nc.scalar.activation(t2[:], dr[:],
                                     mybir.ActivationFunctionType.Copy,
                                     bias=1e-5)
                inv = wpool.tile([128, FW], f32, tag="inv")
                nc.vector.reciprocal(inv[:], t2[:])
                # cutoff = 0.5*sin(pi/6*min(dr,6) + pi/2) + 0.5   (fp16)
                nc.vector.tensor_tensor(
                    t2[:], dr[:],
                    six_sb[:].broadcast_to((128, FW)),
                    op=mybir.AluOpType.min)
                sinv = wpool.tile([128, FW], f16, tag="sinv")
                nc.scalar.activation(sinv[:], t2[:], mybir.ActivationFunctionType.Sin,
                                     bias=_pi2, scale=float(np.pi / R_MAX))
                cut = wpool.tile([128, FW], f16, tag="cut")
                nc.scalar.activation(cut[:], sinv[:],
                                     mybir.ActivationFunctionType.Copy,
                                     bias=0.5, scale=0.5)

                # mono: [128, 20*FW] planes: 1, dnx, dny, dnz, u2(6), u3(10)
                mono = wpool.tile([128, 20 * FW], f16, tag="mono")
                nc.vector.memset(mono[:, 0:FW], 1.0)
                dnx = mono[:, FW:2 * FW]
                dny = mono[:, 2 * FW:3 * FW]
                dnz = mono[:, 3 * FW:4 * FW]
                nc.vector.tensor_tensor(dnx, x, inv[:], op=mybir.AluOpType.mult)
                nc.vector.tensor_tensor(dny, y, inv[:], op=mybir.AluOpType.mult)
                nc.vector.tensor_tensor(dnz, z, inv[:], op=mybir.AluOpType.mult)
                dn3 = mono[:, FW:4 * FW].rearrange("p (c f) -> p c f", c=3)
                # u2 planes 4..9 : xx,xy,xz | yy,yz | zz
                u2v = mono[:, 4 * FW:10 * FW].rearrange("p (c f) -> p c f", c=6)
                nc.gpsimd.tensor_tensor(
                    u2v[:, 0:3, :],
                    dnx.unsqueeze(1).broadcast_to((128, 3, FW)),
                    dn3, op=mybir.AluOpType.mult)
                nc.gpsimd.tensor_tensor(
                    u2v[:, 3:5, :],
                    dny.unsqueeze(1).broadcast_to((128, 2, FW)),
                    dn3[:, 1:3, :], op=mybir.AluOpType.mult)
                nc.gpsimd.tensor_tensor(
                    u2v[:, 5:6, :],
                    dnz.unsqueeze(1).broadcast_to((128, 1, FW)),
                    dn3[:, 2:3, :], op=mybir.AluOpType.mult)
                # u3 planes 10..19
                u3v = mono[:, 10 * FW:20 * FW].rearrange("p (c f) -> p c f", c=10)
                nc.gpsimd.tensor_tensor(
                    u3v[:, 0:3, :],
                    u2v[:, 0:1, :].broadcast_to((128, 3, FW)),
                    dn3, op=mybir.AluOpType.mult)
                nc.gpsimd.tensor_tensor(
                    u3v[:, 3:5, :],
                    u2v[:, 1:2, :].broadcast_to((128, 2, FW)),
                    dn3[:, 1:3, :], op=mybir.AluOpType.mult)
                nc.gpsimd.tensor_tensor(
                    u3v[:, 5:6, :],
                    u2v[:, 2:3, :].broadcast_to((128, 1, FW)),
                    dn3[:, 2:3, :], op=mybir.AluOpType.mult)
                nc.gpsimd.tensor_tensor(
                    u3v[:, 6:8, :],
                    u2v[:, 3:4, :].broadcast_to((128, 2, FW)),
                    dn3[:, 1:3, :], op=mybir.AluOpType.mult)
                nc.gpsimd.tensor_tensor(
                    u3v[:, 8:9, :],
                    u2v[:, 4:5, :].broadcast_to((128, 1, FW)),
                    dn3[:, 2:3, :], op=mybir.AluOpType.mult)
                nc.gpsimd.tensor_tensor(
                    u3v[:, 9:10, :],
                    u2v[:, 5:6, :].broadcast_to((128, 1, FW)),
                    dn3[:, 2:3, :], op=mybir.AluOpType.mult)

                # basis: g = exp(-betta*(dr-s)^2)  [128, 7*FW] fp16
                diff = wpool.tile([128, N_BASIS * FW], f16, tag="diff")
                dv = diff[:].rearrange("p (b f) -> p b f", b=N_BASIS)
                nc.vector.tensor_tensor(
                    dv,
                    dr[:].unsqueeze(1).broadcast_to((128, N_BASIS, FW)),
                    shifts_sb[:].unsqueeze(2).broadcast_to((128, N_BASIS, FW)),
                    op=mybir.AluOpType.subtract)
                g = wpool.tile([128, N_BASIS * FW], f16, tag="g")
                gv = g[:].rearrange("p (b f) -> p b f", b=N_BASIS)
                nc.vector.tensor_tensor(gv, dv, dv, op=mybir.AluOpType.mult)
                nc.scalar.activation(g[:], g[:], mybir.ActivationFunctionType.Exp,
                                     scale=-BETTA)

                # rad[r] = cutoff * sum_b coef[r,b]*g[b]    [128, 5*FW] fp16
                coefv = coef_sb[:].rearrange("p (r b f) -> p r b f", r=N_RADIAL, b=N_BASIS)
                radA = wpool.tile([128, N_RADIAL * FW], f16, tag="radA")
                radB = wpool.tile([128, N_RADIAL * FW], f16, tag="radB")
                rAv = radA[:].rearrange("p (r f) -> p r f", r=N_RADIAL)
                rBv = radB[:].rearrange("p (r f) -> p r f", r=N_RADIAL)
                for b in range(N_BASIS):
                    gb = gv[:, b, :].unsqueeze(1).broadcast_to((128, N_RADIAL, FW))
                    tgt = rAv if b == 0 else rBv
                    nc.vector.tensor_tensor(tgt, coefv[:, :, b, :], gb,
                                            op=mybir.AluOpType.mult)
                    if b > 0:
                        nc.vector.tensor_tensor(rAv, rAv, rBv, op=mybir.AluOpType.add)
                radf = wpool.tile([128, N_RADIAL * FW], f16, tag="radf")
                rfv = radf[:].rearrange("p (r f) -> p r f", r=N_RADIAL)
                nc.vector.tensor_tensor(
                    rfv, rAv,
                    cut[:].unsqueeze(1).broadcast_to((128, N_RADIAL, FW)),
                    op=mybir.AluOpType.mult)

                # medge [128, 100*FW] = rad (r) x mono (t)
                medge = mpool.tile([128, NCOMP * FW], f16, tag="medge")
                mev = medge[:].rearrange("p (r t f) -> p r t f", r=N_RADIAL, t=20)
                nc.vector.tensor_tensor(
                    mev,
                    rfv.unsqueeze(2).broadcast_to((128, N_RADIAL, 20, FW)),
                    mono[:].rearrange("p (t f) -> p t f", t=20)
                        .unsqueeze(1).broadcast_to((128, N_RADIAL, 20, FW)),
                    op=mybir.AluOpType.mult)

                # onehot [128, FW*WA]
                oh = mpool.tile([128, FW * WA], f16, tag="oh")
                ohv = oh[:].rearrange("p (f c) -> p f c", c=WA)
                oh4 = oh[:].rearrange("p (f c q) -> p f c q", c=WA // 2, q=2)
                nc.vector.tensor_tensor(
                    oh4,
                    iota_sb[:].rearrange("p (c q) -> p c q", q=2)
                        .unsqueeze(1).broadcast_to((128, FW, WA // 2, 2)),
                    al_sb[:].rearrange("p (f q) -> p f q", q=2)
                        .unsqueeze(2).broadcast_to((128, FW, WA // 2, 2)),
                    op=mybir.AluOpType.is_equal)

                # segment-sum matmuls: 4 windows in this chunk
                mev2 = medge[:].rearrange("p (c f) -> p c f", c=NCOMP)
                for w4 in range(CPW):
                    w = ch * CPW + w4
                    pair = w // 2
                    hi = w % 2
                    if hi == 0:
                        ps = pspool.tile([128, NCOMP], f32, tag="ps")
                    for b in range(BPW):
                        col = w4 * BPW + b
                        nc.tensor.matmul(
                            ps[hi * WA:(hi + 1) * WA, :],
                            ohv[:, col, :],
                            mev2[:, :, col],
                            start=(b == 0), stop=(b == BPW - 1),
                            tile_position=(0, hi * WA),
                        )
                    if hi == 1:
                        nc.scalar.copy(
                            M_sb[:, pair * NCOMP:(pair + 1) * NCOMP], ps[:, :])

            # ----------------- phase B: per-atom contractions -----------------
            _lp = nc.allow_low_precision("f16 phase-B scratch; final out is f32")
            _lp.__enter__()
            # NOTE: TensorTensor ISA allows at most 3 free dims per operand and
            # broadcast dims never merge -> split every 4-dim product in python.
            T = T_TILES
            Mv = M_sb[:].rearrange("p (T r q) -> p T r q", T=T, r=N_RADIAL)
            m1 = Mv[:, :, :, 1:4]
            s2 = Mv[:, :, :, 4:10]
            s3 = Mv[:, :, :, 10:20]

            out_sb = bpool.tile([128, T * NOUT], f32)
            outv = out_sb[:].rearrange("p (T c) -> p T c", T=T)

            mul = mybir.AluOpType.mult
            addop = mybir.AluOpType.add
            X = mybir.AxisListType.X

            # m0
            nc.vector.tensor_copy(outv[:, :, 0:5], Mv[:, :, :, 0])

            # weighted s2w, s3w
            s2w = bpool.tile([128, T * 5 * 6], f16)
            s2wv = s2w[:].rearrange("p (T r u) -> p T r u", T=T, r=5)
            nc.vector.tensor_tensor(
                s2wv, s2,
                wc_sb[:, 0:6].unsqueeze(1).unsqueeze(1).broadcast_to((128, T, 5, 6)),
                op=mul)
            s3w = bpool.tile([128, T * 5 * 10], f16)
            s3wv = s3w[:].rearrange("p (T r u) -> p T r u", T=T, r=5)
            nc.vector.tensor_tensor(
                s3wv, s3,
                wc_sb[:, 6:16].unsqueeze(1).unsqueeze(1).broadcast_to((128, T, 5, 10)),
                op=mul)

            # c1/c2/c3 grams (split per s)
            for (lhs, rhs_, width, off) in ((m1, m1, 3, 5), (s2wv, s2, 6, 30), (s3wv, s3, 10, 55)):
                for s in range(5):
                    pb = spool.tile([128, T * 5 * 10], f16, tag="pb")
                    pv = pb[:, 0:T * 5 * width].rearrange(
                        "p (T r u) -> p T r u", T=T, r=5)
                    nc.vector.tensor_tensor(
                        pv, lhs,
                        rhs_[:, :, s, :].unsqueeze(2).broadcast_to((128, T, 5, width)),
                        op=mul)
                    nc.vector.tensor_reduce(
                        outv[:, :, off + s:off + 25:5], pv, axis=X, op=addop)

            # m2full [128, T*5*9]
            m2f = bpool.tile([128, T * 5 * 9], f16)
            m2fv = m2f[:].rearrange("p (T r c) -> p T r c", T=T, r=5)
            for (d0, s0, st, n) in _affine_runs(M2MAP):
                nc.vector.tensor_copy(m2fv[:, :, :, d0:d0 + n],
                                      s2[:, :, :, s0:s0 + st * (n - 1) + 1:st] if st != 0 else
                                      s2[:, :, :, s0:s0 + 1].broadcast_to((128, T, 5, n)))
            # m3full [128, T*5*27]
            m3f = bpool.tile([128, T * 5 * 27], f16)
            m3fv = m3f[:].rearrange("p (T r c) -> p T r c", T=T, r=5)
            for (d0, s0, st, n) in _affine_runs(M3MAP):
                nc.vector.tensor_copy(m3fv[:, :, :, d0:d0 + n],
                                      s3[:, :, :, s0:s0 + st * (n - 1) + 1:st] if st != 0 else
                                      s3[:, :, :, s0:s0 + 1].broadcast_to((128, T, 5, n)))

            # c5 (cols 80:155): c5[r,s,t] = sum_ij m1[r,i] m1[s,j] m2f[t,ij]
            for r in range(5):
                for s in range(r + 1):
                    pair = r * (r + 1) // 2 + s
                    p1b = spool.tile([128, T * 9], f16, tag="p1b")
                    qb = spool.tile([128, T * 5 * 9], f16, tag="qb")
                    p1v = p1b[:].rearrange("p (T u) -> p T u", T=T)
                    p1v9 = p1b[:].rearrange("p (T i j) -> p T i j", T=T, i=3)
                    nc.vector.tensor_tensor(
                        p1v9,
                        m1[:, :, r, :].unsqueeze(3).broadcast_to((128, T, 3, 3)),
                        m1[:, :, s, :].unsqueeze(2).broadcast_to((128, T, 3, 3)),
                        op=mul)
                    qv = qb[:].rearrange("p (T t u) -> p T t u", T=T, t=5)
                    nc.vector.tensor_tensor(
                        qv,
                        p1v.unsqueeze(2).broadcast_to((128, T, 5, 9)),
                        m2fv, op=mul)
                    nc.vector.tensor_reduce(
                        outv[:, :, 80 + pair * 5:80 + pair * 5 + 5],
                        qv, axis=X, op=addop)

            # c4 (cols 155:230): A_rs(j,k) = sum_i m2f[r](i,j) m2f[s](i,k)
            for r in range(5):
                m2fr = m2fv[:, :, r, :].rearrange("p T (i j) -> p T i j", i=3)
                for s in range(r + 1):
                    pair = r * (r + 1) // 2 + s
                    ab = spool.tile([128, T * 9], f16, tag="ab")
                    abv = ab[:].rearrange("p (T j k) -> p T j k", T=T, j=3)
                    qb = spool.tile([128, T * 5 * 9], f16, tag="qb")
                    m2fs = m2fv[:, :, s, :].rearrange("p T (i k) -> p T i k", i=3)
                    for j in range(3):
                        tb = spool.tile([128, T * 27], f16, tag="tb")
                        tv = tb[:, 0:T * 3 * 3].rearrange(
                            "p (T k i) -> p T k i", T=T, k=3)
                        nc.gpsimd.tensor_tensor(
                            tv,
                            m2fr[:, :, :, j].unsqueeze(2).broadcast_to((128, T, 3, 3)),
                            m2fs.rearrange("p T i k -> p T k i"),
                            op=mul)
                        nc.vector.tensor_reduce(
                            abv[:, :, j, :], tv, axis=X, op=addop)
                    qv = qb[:].rearrange("p (T t u) -> p T t u", T=T, t=5)
                    nc.vector.tensor_tensor(
                        qv,
                        ab[:].rearrange("p (T u) -> p T u", T=T)
                            .unsqueeze(2).broadcast_to((128, T, 5, 9)),
                        m2fv, op=mul)
                    nc.vector.tensor_reduce(
                        outv[:, :, 155 + pair * 5:155 + pair * 5 + 5],
                        qv, axis=X, op=addop)

            # c6 (cols 230:305): B_rs(k,l) = sum_ij m3f[r](ij,k) m3f[s](ij,l)
            for r in range(5):
                m3fr = m3fv[:, :, r, :].rearrange("p T (u k) -> p T u k", u=9)
                for s in range(r + 1):
                    pair = r * (r + 1) // 2 + s
                    ab = spool.tile([128, T * 9], f16, tag="ab")
                    abv = ab[:].rearrange("p (T j k) -> p T j k", T=T, j=3)
                    qb = spool.tile([128, T * 5 * 9], f16, tag="qb")
                    m3fs = m3fv[:, :, s, :].rearrange("p T (u l) -> p T u l", u=9)
                    for k in range(3):
                        tb = spool.tile([128, T * 27], f16, tag="tb")
                        tv = tb[:, 0:T * 3 * 9].rearrange(
                            "p (T l u) -> p T l u", T=T, l=3)
                        nc.gpsimd.tensor_tensor(
                            tv,
                            m3fr[:, :, :, k].unsqueeze(2).broadcast_to((128, T, 3, 9)),
                            m3fs.rearrange("p T u l -> p T l u"),
                            op=mul)
                        nc.vector.tensor_reduce(
                            abv[:, :, k, :], tv, axis=X, op=addop)
                    qv = qb[:].rearrange("p (T t u) -> p T t u", T=T, t=5)
                    nc.vector.tensor_tensor(
                        qv,
                        ab[:].rearrange("p (T u) -> p T u", T=T)
                            .unsqueeze(2).broadcast_to((128, T, 5, 9)),
                        m2fv, op=mul)
                    nc.vector.tensor_reduce(
                        outv[:, :, 230 + pair * 5:230 + pair * 5 + 5],
                        qv, axis=X, op=addop)

            # c7 (cols 305:430): C_rs(k) = sum_ij m3f[r](ij,k) m2f[s](ij)
            for r in range(5):
                m3fr = m3fv[:, :, r, :].rearrange("p T (u k) -> p T k u", u=9)
                for s in range(5):
                    tb = spool.tile([128, T * 27], f16, tag="tb")
                    cb = spool.tile([128, T * 5 * 3], f16, tag="cb")
                    qb = spool.tile([128, T * 5 * 9], f16, tag="qb")
                    tv = tb[:, 0:T * 3 * 9].rearrange(
                        "p (T k u) -> p T k u", T=T, k=3)
                    nc.gpsimd.tensor_tensor(
                        tv, m3fr,
                        m2fv[:, :, s, :].unsqueeze(2).broadcast_to((128, T, 3, 9)),
                        op=mul)
                    nc.vector.tensor_reduce(
                        cb[:, 0:T * 3].rearrange("p (T k) -> p T k", T=T),
                        tv, axis=X, op=addop)
                    qv = qb[:, 0:T * 5 * 3].rearrange(
                        "p (T t k) -> p T t k", T=T, t=5)
                    nc.vector.tensor_tensor(
                        qv,
                        cb[:, 0:T * 3].rearrange("p (T k) -> p T k", T=T)
                            .unsqueeze(2).broadcast_to((128, T, 5, 3)),
                        m1, op=mul)
                    nc.vector.tensor_reduce(
                        outv[:, :, 305 + r * 25 + s * 5:305 + r * 25 + s * 5 + 5],
                        qv, axis=X, op=addop)

            nc.sync.dma_start(out_d[:, :], out_sb[:])
            _lp.__exit__(None, None, None)

    nc.compile()
    return nc


_CACHE = {}


def kernel(dr_vec, Z, neighbor_idxs, W):
    from concourse.bass_utils import run_bass_kernel_spmd

    ins, meta = _prep(dr_vec, Z, neighbor_idxs, W)
    BPW = meta["BPW"]
    if BPW not in _CACHE:
        _CACHE[BPW] = _build_program(BPW)
    nc = _CACHE[BPW]

    in_maps = []
    for c in range(N_CORE):
        in_maps.append({
            "xyz": ins["xyz"][c], "coef": ins["coef"][c], "al": ins["al"][c],
            "iota": ins["iota"], "shifts": ins["shifts"], "wconst": ins["wconst"],
        })
    global LAST_EXEC_NS, LAST_RUN_WALL
    _t0 = __import__("time").perf_counter()
    trace = bool(int(os.environ.get("BASS_KERNEL_TRACE", "0")))
    res = run_bass_kernel_spmd(nc, in_maps, core_ids=list(range(N_CORE)),
                               trace=trace)
    LAST_RUN_WALL = __import__("time").perf_counter() - _t0
    LAST_EXEC_NS = res.exec_time_ns

    # assemble
    atoms_of_slot = meta["atoms_of_slot"]  # [NW, WA]
    O = np.zeros((A_PAD, NOUT), dtype=np.float32)
    for c in range(N_CORE):
        o = res.results[c]["out"].reshape(128, T_TILES, NOUT)
        o = np.transpose(o, (1, 0, 2)).reshape(APC, NOUT)
        O[c * APC:(c + 1) * APC] = o

    full = np.zeros((N_ATOMS, NOUT), dtype=np.float32)
    slots = atoms_of_slot.reshape(-1)
    valid = slots >= 0
    full[slots[valid]] = O[valid]

    t2 = _tril_2d(N_RADIAL)
    t3 = _tril_3d(N_RADIAL)
    pair_idx = (t2[:, 0] * (t2[:, 0] + 1)) // 2 + t2[:, 1]
    out = np.empty((N_ATOMS, 360), dtype=np.float32)
    out[:, 0:5] = full[:, 0:5]
    c1 = full[:, 5:30].reshape(-1, 5, 5)
    c2 = full[:, 30:55].reshape(-1, 5, 5)
    c3 = full[:, 55:80].reshape(-1, 5, 5)
    out[:, 5:20] = c1[:, t2[:, 0], t2[:, 1]]
    out[:, 20:35] = c2[:, t2[:, 0], t2[:, 1]]
    out[:, 35:50] = c3[:, t2[:, 0], t2[:, 1]]
    c4 = full[:, 155:230].reshape(-1, 15, 5)
    p3 = (t3[:, 0] * (t3[:, 0] + 1)) // 2 + t3[:, 1]
    out[:, 50:85] = c4[:, p3, t3[:, 2]]
    out[:, 85:160] = full[:, 80:155]
    out[:, 160:235] = full[:, 230:305]
    out[:, 235:360] = full[:, 305:430]
    return out
